# revision 20
# baseline (speedup 1.0000x reference)
"""BloomAttention Trainium2 kernel (interleaved-phase schedule).

Reference semantics (B=2, S=2048, H=2048, NH=16, HD=128):
  mixed = hs @ w_qkv.T + b_qkv, reshaped [b,s,nh,3hd] then reinterpreted
  Megatron-style as (s, b*nh, hd).  With B=2 that reinterpretation scrambles
  (batch, position) into 32 independent "virtual sequences" indexed by
  (parity p, head n): virtual seq (p, n) consists of flat tokens
  t = 2*s' + p (t = b*S + s_pos) in increasing s' order.  Attention (with
  alibi[n, k'] bias, causal mask over virtual positions, softmax) runs per
  virtual sequence; the dense projection maps back so that
  out[p, s', :] = dense(concat_n ctx_{p,n}[s']).

Sharding: 2 heads per core (Megatron column-split of w_qkv, row-split of
w_dense), both parities; host sums the 8 partial dense outputs.

Device layouts (per core c, heads {2c, 2c+1}):
  hsr  [2048h, 4096t']   t' = p*2048 + s'  (host pre-transposed/reordered)
  qk   [512j, 4096t']    j = [q0,k0,q1,k1] blocks of 128   (= mixed.T slice)
  v    [4096t', 256c']   c' = (n_l, d)
  scores S.T [k', s'] per vseq; P = exp(S/sqrt(HD) + alibi) * causal01
  ctx.T [128d, s'] per (vseq);  den via ones-matmul;  dense out [t', 2048].

Schedule: attention kt-steps are software-pipelined against QKV / dense
matmul "filler" units at instruction granularity (generator-driven emission),
so the in-order TensorE queue never stalls on ScalarE exp.  The softmax
denominator for full (non causal-diagonal) groups of 4 k-tiles is computed by
summing P tiles pairwise on DVE (bf16 all-SBUF = fast mode) and doing one
ones-matmul per group; diagonal tiles keep per-tile den matmuls.  QKV runs as
4 passes per token-block (q01/v01/v23/q23, 2 psum tiles each) so attention
blocks can start as soon as their q/k/v passes have landed; block (0,0)
emits its scores right after pass q01 to cover the initial hs-DMA latency.
Early DMAs are spread across the sync/scalar/gpsimd queues with the
late-needed halves (q1k1 weight columns, w_dense) deferred.  PSUM: 5-bank
flex ring shared by QKV passes, score tiles and dense tiles (kills
pass-transition bias-add stalls), 2 ctx banks, 1 den bank.  Dense psum is
copied out on DVE (hb0/1, +hb2 for p1) and ScalarE, with per-half-row DMAs
split across the sync and gpsimd queues.
"""

import math
import os
import sys
from collections import deque

for _p in ("/opt/trn_rl_repo", "/root/.axon_site/_ro/trn_rl_repo"):
    if os.path.isdir(_p) and _p not in sys.path:
        sys.path.append(_p)

import numpy as np
import ml_dtypes
import concourse.bass as bass
import concourse.tile as tile
from concourse import mybir, bacc
from concourse.bass_utils import run_bass_kernel_spmd

F32 = mybir.dt.float32
BF16 = mybir.dt.bfloat16
AF = mybir.ActivationFunctionType

B, S, H, NH = 2, 2048, 2048, 16
HD = H // NH
T = B * S                  # 4096 flat tokens
NHT = H // 128             # 16 h-tiles
JQK = 4 * 128              # local q+k rows
JV = 2 * 128               # local v rows
NTB = T // 512             # 8 token-blocks
NKT = S // 128             # 16 key tiles per virtual sequence
NSB = S // 512             # 4 query blocks per virtual sequence
INV_SQRT_HD = 1.0 / math.sqrt(HD)

_cache = {}


def _build_nc():
    nc = bacc.Bacc()
    hsr = nc.declare_dram_parameter("hsr", [H, T], BF16, isOutput=False)
    wqkT = nc.declare_dram_parameter("wqkT", [H, JQK], BF16, isOutput=False)
    wvT = nc.declare_dram_parameter("wvT", [H, JV], BF16, isOutput=False)
    wdT = nc.declare_dram_parameter("wdT", [JV, H], BF16, isOutput=False)
    bqk = nc.declare_dram_parameter("bqk", [JQK], F32, isOutput=False)
    bvbc = nc.declare_dram_parameter("bvbc", [128, JV], F32, isOutput=False)
    albt = nc.declare_dram_parameter("albt", [128, 2, NKT], F32, isOutput=False)
    mskt = nc.declare_dram_parameter("mskt", [128, 896], BF16, isOutput=False)
    part = nc.declare_dram_parameter("part", [T, H], BF16, isOutput=True)

    with tile.TileContext(nc) as tc:
        with (
            tc.tile_pool(name="consts", bufs=1) as consts,
            tc.tile_pool(name="wpool", bufs=1) as wpool,
            tc.tile_pool(name="hsrp", bufs=1) as hsrp,
            tc.tile_pool(name="qkvout", bufs=1) as qkvout,
            tc.tile_pool(name="ptp", bufs=1) as ptp,
            tc.tile_pool(name="gsp", bufs=1) as gsp,
            tc.tile_pool(name="smallp", bufs=1) as smallp,
            tc.tile_pool(name="ctxp", bufs=1) as ctxp,
            tc.tile_pool(name="outsbp", bufs=1) as outsbp,
            tc.tile_pool(name="pflex", bufs=5, space="PSUM") as pflex,
            tc.tile_pool(name="ppctx", bufs=2, space="PSUM") as ppctx,
            tc.tile_pool(name="ppden", bufs=1, space="PSUM") as ppden,
        ):
            # ---------------- constants / weights ----------------
            # the q0/k0 half (j 0:256) feeds pass q01 immediately; the q1/k1
            # half is not needed until pass q23 — load it after the consts
            wqk_big = []
            for hg in range(4):
                wq_t = wpool.tile([128, 4, JQK], BF16, tag=f"wqk{hg}",
                                  name=f"wqk{hg}")
                if hg == 0:
                    for j in range(4):
                        nc.sync.dma_start(
                            out=wq_t[:, j, 0:256],
                            in_=wqkT[j * 128:(j + 1) * 128, 0:256])
                else:
                    nc.sync.dma_start(
                        out=wq_t[:, :, 0:256],
                        in_=wqkT[hg * 512:(hg + 1) * 512, 0:256].rearrange(
                            "(j p) f -> p j f", p=128))
                wqk_big.append(wq_t)

            def emit_wqk_late_loads():
                for hg in range(4):
                    nc.sync.dma_start(
                        out=wqk_big[hg][:, :, 256:512],
                        in_=wqkT[hg * 512:(hg + 1) * 512, 256:512].rearrange(
                            "(j p) f -> p j f", p=128))
            wv_big = []

            def emit_wv_loads(hgs=range(4)):
                for hg in hgs:
                    wv_t = wpool.tile([128, 4, JV], BF16, tag=f"wv{hg}",
                                      name=f"wv{hg}")
                    nc.scalar.dma_start(
                        out=wv_t,
                        in_=wvT[hg * 512:(hg + 1) * 512, :].rearrange(
                            "(j p) f -> p j f", p=128))
                    wv_big.append(wv_t)

            bqk_sb = consts.tile([128, 4], F32)
            nc.sync.dma_start(out=bqk_sb, in_=bqk.rearrange("(jt p) -> p jt", p=128))
            bv_bc = consts.tile([128, JV], F32)
            nc.sync.dma_start(out=bv_bc, in_=bvbc[:, :])
            alb_sb = consts.tile([128, 2, NKT], F32)
            nc.sync.dma_start(out=alb_sb, in_=albt[:, :, :])
            mask_sb = consts.tile([128, 896], BF16)
            nc.sync.dma_start(out=mask_sb, in_=mskt[:, :])
            # carved from the template: tri[p, c] = (c >= p); ones128 all-ones
            tri128 = mask_sb[:, 384:512]
            ones128 = mask_sb[:, 512:640]

            emit_wqk_late_loads()

            wd_sb = consts.tile([128, 2, H], BF16)

            def wqk_t(ht):
                return wqk_big[ht // 4][:, ht % 4, :]

            def wv_tt(ht):
                return wv_big[ht // 4][:, ht % 4, :]

            # ---------------- hs tile loads ----------------
            hs_tiles = {}  # tb -> list of 4 [128,4,512] tiles

            def emit_hs_loads(tb, split_first=False, eng=None, eng2=None):
                eng = eng or nc.gpsimd
                hbig = []
                for hg in range(4):
                    e = eng2 if (eng2 is not None and hg >= 2) else eng
                    h_t = hsrp.tile([128, 4, 512], BF16, tag="hsr", bufs=8,
                                    name=f"hsr{tb}_{hg}")
                    if split_first and hg == 0:
                        for j in range(4):
                            e.dma_start(
                                out=h_t[:, j, :],
                                in_=hsr[j * 128:(j + 1) * 128,
                                        tb * 512:(tb + 1) * 512])
                    else:
                        e.dma_start(
                            out=h_t,
                            in_=hsr[hg * 512:(hg + 1) * 512,
                                    tb * 512:(tb + 1) * 512].rearrange(
                                        "(j p) f -> p j f", p=128))
                    hbig.append(h_t)
                hs_tiles[tb] = hbig

            def hs_t(tb, ht):
                return hs_tiles[tb][ht // 4][:, ht % 4, :]

            qk_sb = {}  # (jt, tb) -> [128, 512] tile
            v_sb = {}   # tt (0..15) -> [128, 256] tile

            # ---------------- QKV generator (filler units) ----------------
            def qkv_gen(tb):
                # passes: ('q', jt0, jt1) into 512-wide psum; ('v', tt0, tt1)
                # into 256-wide psum.  2 banks (tags a/b) ping-pong them all.
                for pi, (kind, i0, i1) in enumerate(
                        [('q', 0, 1), ('v', 0, 1), ('v', 2, 3), ('q', 2, 3)]):
                    width = 512 if kind == 'q' else JV
                    pA = pflex.tile([128, 512], F32, tag="flex",
                                    name=f"p{kind}{tb}_{i0}")[:, :width]
                    pB = pflex.tile([128, 512], F32, tag="flex",
                                    name=f"p{kind}{tb}_{i1}")[:, :width]
                    for ht in range(NHT):
                        st = ht == 0
                        sp = ht == NHT - 1
                        if kind == 'q':
                            nc.tensor.matmul(
                                pA, lhsT=wqk_t(ht)[:, i0 * 128:(i0 + 1) * 128],
                                rhs=hs_t(tb, ht), start=st, stop=sp)
                            nc.tensor.matmul(
                                pB, lhsT=wqk_t(ht)[:, i1 * 128:(i1 + 1) * 128],
                                rhs=hs_t(tb, ht), start=st, stop=sp)
                        else:
                            nc.tensor.matmul(
                                pA, lhsT=hs_t(tb, ht)[:, i0 * 128:(i0 + 1) * 128],
                                rhs=wv_tt(ht), start=st, stop=sp)
                            nc.tensor.matmul(
                                pB, lhsT=hs_t(tb, ht)[:, i1 * 128:(i1 + 1) * 128],
                                rhs=wv_tt(ht), start=st, stop=sp)
                        # prefetch hs two blocks ahead (tb+1 was loaded by
                        # the previous gen / the initial loads)
                        if pi == 1 and ht == 8 and tb + 2 < NTB:
                            emit_hs_loads(tb + 2)
                        yield
                    if kind == 'q':
                        for jt, pp in ((i0, pA), (i1, pB)):
                            qt = qkvout.tile([128, 512], BF16,
                                             tag=f"qk{jt}_{tb}",
                                             name=f"qk{jt}_{tb}")
                            nc.vector.tensor_scalar_add(qt, pp,
                                                        bqk_sb[:, jt:jt + 1])
                            qk_sb[(jt, tb)] = qt
                    else:
                        for tt, pp in ((i0, pA), (i1, pB)):
                            vt = qkvout.tile([128, JV], BF16,
                                             tag=f"v{tb * 4 + tt}",
                                             name=f"v{tb * 4 + tt}")
                            nc.vector.tensor_add(vt, pp, bv_bc)
                            v_sb[tb * 4 + tt] = vt
                    yield "p"

            # ---------------- filler machinery ----------------
            fillers = deque()
            pass_counts = {}

            def step(gen):
                v = next(gen)
                if v:
                    pass_counts[gen] = pass_counts.get(gen, 0) + 1

            def feed(n=1):
                while n > 0 and fillers:
                    try:
                        step(fillers[0])
                        n -= 1
                    except StopIteration:
                        fillers.popleft()

            def drain(gen, to_pass=None):
                while to_pass is None or pass_counts.get(gen, 0) < to_pass:
                    try:
                        step(gen)
                    except StopIteration:
                        break
                else:
                    return  # partial drain keeps gen in the deque
                if fillers and fillers[0] is gen:
                    fillers.popleft()
                elif gen in fillers:
                    fillers.remove(gen)

            # ---------------- attention block ----------------
            ctx_tiles = {}

            def st_exp_one(p, nl, b, kt, pts):
                off = max(0, 128 * (kt - 4 * b))
                pst = pflex.tile([128, 512], F32, tag="flex",
                                 name=f"pst{p}{nl}{b}_{kt}")
                ktile = qk_sb[(2 * nl + 1, p * 4 + kt // 4)]
                q_rhs = qk_sb[(2 * nl, p * 4 + b)]
                nc.tensor.matmul(
                    pst[:, off:],
                    lhsT=ktile[:, (kt % 4) * 128:(kt % 4 + 1) * 128],
                    rhs=q_rhs[:, off:],
                    start=True, stop=True,
                )
                pt = ptp.tile([128, 512], BF16, tag="pt", bufs=8,
                              name=f"pt{p}{nl}{b}_{kt}")
                nc.scalar.activation(out=pt[:, off:], in_=pst[:, off:],
                                     func=AF.Exp,
                                     bias=alb_sb[:, nl, kt:kt + 1],
                                     scale=INV_SQRT_HD)
                if kt >= 4 * b:
                    nc.vector.tensor_mul(
                        pt[:, off:off + 128], pt[:, off:off + 128], tri128)
                pts[kt] = pt

            def attn_block(p, nl, b, pre=None):
                nkt = 4 * b + 4
                ctx_t = ctx_tiles[(p, b)]
                pctx = ppctx.tile([128, 512], F32, tag="ctx",
                                  name=f"pctx{p}{nl}{b}")
                pden = ppden.tile([128, 512], F32, tag="den",
                                  name=f"pden{p}{nl}{b}")
                pts = pre if pre is not None else {}
                n_pre = len(pts)
                gparts = {}
                state = {"den_started": False}

                def off_of(kt):
                    return max(0, 128 * (kt - 4 * b))

                def st_exp(kt):
                    if kt < n_pre:
                        return
                    st_exp_one(p, nl, b, kt, pts)

                def den_mm(rhs_ap, off, stop):
                    nc.tensor.matmul(
                        pden[:, off:], lhsT=ones128, rhs=rhs_ap,
                        start=not state["den_started"], stop=stop,
                    )
                    state["den_started"] = True

                st_exp(0)
                for kt in range(nkt):
                    if kt + 1 < nkt:
                        st_exp(kt + 1)
                    # ctx accumulation
                    off = off_of(kt)
                    vtile = v_sb[p * 16 + kt]
                    nc.tensor.matmul(
                        pctx[:, off:],
                        lhsT=vtile[:, nl * 128:(nl + 1) * 128],
                        rhs=pts[kt][:, off:],
                        start=(kt == 0), stop=(kt == nkt - 1),
                    )
                    # denominator
                    if kt >= 4 * b:
                        # causal-diagonal tiles: per-kt den matmul
                        den_mm(pts[kt][:, off:], off, stop=(kt == nkt - 1))
                    else:
                        # full tiles: DVE pair sums, one den matmul per 4
                        if kt % 2 == 1:
                            half = (kt % 4) // 2
                            gp = gsp.tile([128, 512], BF16,
                                          tag=f"gp{half}", bufs=2,
                                          name=f"gp{p}{nl}{b}_{kt}")
                            nc.vector.tensor_add(gp, pts[kt - 1], pts[kt])
                            gparts[half] = gp
                        if kt % 4 == 3:
                            gs = gsp.tile([128, 512], BF16, tag="gs", bufs=2,
                                          name=f"gs{p}{nl}{b}_{kt}")
                            nc.vector.tensor_add(gs, gparts[0], gparts[1])
                            den_mm(gs, 0, stop=False)
                    if kt >= 2:
                        del pts[kt - 2]
                    feed(2)

                bc = smallp.tile([128, 512], F32, tag="bcast", bufs=2,
                                 name=f"bc{p}{nl}{b}")
                nc.vector.reciprocal_approx_fast(out=bc, in_=pden)
                nc.vector.tensor_mul(ctx_t[:, nl, :], pctx, bc)

            # ---------------- dense generator (filler units) ----------------
            def dense_gen(p, b):
                ctx_t = ctx_tiles[(p, b)]
                for i in range(4):
                    tt = p * 16 + b * 4 + i
                    ot = outsbp.tile([128, H], BF16, tag="outsb", bufs=10,
                                     name=f"ot{tt}")
                    dma_engs = ([nc.sync, nc.gpsimd, nc.sync, nc.scalar]
                                if p == 1 else
                                [nc.sync, nc.gpsimd, nc.gpsimd, nc.sync])
                    for half in range(2):
                        for hb in (2 * half, 2 * half + 1):
                            po = pflex.tile([128, 512], F32, tag="flex",
                                            name=f"po{tt}_{hb}")
                            for nl in range(2):
                                nc.tensor.matmul(
                                    po,
                                    lhsT=ctx_t[:, nl, i * 128:(i + 1) * 128],
                                    rhs=wd_sb[:, nl, hb * 512:(hb + 1) * 512],
                                    start=(nl == 0), stop=(nl == 1),
                                )
                            dst = ot[:, hb * 512:(hb + 1) * 512]
                            if hb < 2 or (p == 1 and hb == 2):
                                nc.vector.tensor_copy(out=dst, in_=po)
                            else:
                                nc.scalar.activation(out=dst, in_=po,
                                                     func=AF.Copy)
                            dma_engs[hb].dma_start(
                                out=part[tt * 128:(tt + 1) * 128,
                                         hb * 512:(hb + 1) * 512],
                                in_=dst)
                        yield

            # ---------------- schedule ----------------
            emit_hs_loads(0, split_first=True, eng2=nc.scalar)
            emit_wv_loads()
            emit_hs_loads(1)
            # w_dense is needed last; keep it at the back of the scalar queue
            for nl in range(2):
                nc.scalar.dma_start(out=wd_sb[:, nl, :],
                                    in_=wdT[nl * 128:(nl + 1) * 128, :])

            qkv_gens = {}
            for t in range(NTB):
                qkv_gens[t] = qkv_gen(t)
                fillers.append(qkv_gens[t])

            # attention slots: (p, b, qkv tb whose tiles gate the block)
            slots = [(0, 0, 0), (0, 1, 1), (0, 2, 2), (0, 3, 3),
                     (1, 0, 4), (1, 1, 5), (1, 2, 6), (1, 3, 7)]
            for p, b, dep in slots:
                for t in range(dep):
                    if t in qkv_gens:
                        drain(qkv_gens[t])
                        del qkv_gens[t]
                ctx_tiles[(p, b)] = ctxp.tile([128, 2, 512], BF16,
                                              tag=f"ctx{p}{b}",
                                              name=f"ctx{p}{b}")
                # nl=0 needs q01+v01+v23 of gen(dep); nl=1 additionally q23
                gd = qkv_gens.get(dep)
                gnext = qkv_gens.get(dep + 1)
                pre = None
                if (p, b) == (0, 0):
                    # scores need no V: emit them right after pass q01 so the
                    # TensorE has work while the v passes wait on hs DMAs
                    if gd is not None:
                        drain(gd, to_pass=1)
                    pre = {}
                    for kt in range(4):
                        st_exp_one(0, 0, 0, kt, pre)
                if gd is not None:
                    drain(gd, to_pass=3)
                # cover the gating v-bias DVE latency with independent work
                for _ in range(6):
                    if gnext is None:
                        break
                    try:
                        step(gnext)
                    except StopIteration:
                        break
                attn_block(p, 0, b, pre=pre)
                if gd is not None:
                    drain(gd)
                    del qkv_gens[dep]
                feed(6)  # cover the q23-bias latency before nl=1's scores
                attn_block(p, 1, b)
                fillers.append(dense_gen(p, b))

            # drain the last block's dense first (its DMAs are the tail),
            # then round-robin the rest so copies pipeline across engines
            last_dg = fillers[-1]
            drain(last_dg)
            while fillers:
                try:
                    step(fillers[0])
                except StopIteration:
                    fillers.popleft()
                    continue
                fillers.rotate(-1)

    nc.finalize()
    return nc


def _host_prep(inputs):
    hs = np.asarray(inputs["hidden_states"], dtype=np.float32)
    alibi = np.asarray(inputs["alibi"], dtype=np.float32)
    w_qkv = np.asarray(inputs["w_qkv"], dtype=np.float32)
    b_qkv = np.asarray(inputs["b_qkv"], dtype=np.float32)
    w_dense = np.asarray(inputs["w_dense"], dtype=np.float32)

    hs_flat = hs.reshape(T, H)
    # hsr[h, p*S + s'] = hs_flat[2 s' + p, h]
    hsr = np.ascontiguousarray(
        hs_flat.reshape(S, 2, H).transpose(2, 1, 0).reshape(H, T))

    # causal template: M[p, x] = 1 if (x - 384) >= p
    xs = np.arange(896, dtype=np.int64)[None, :] - 384
    ps = np.arange(128, dtype=np.int64)[:, None]
    mskt = (xs >= ps).astype(ml_dtypes.bfloat16)

    w3 = w_qkv.reshape(NH, 3 * HD, H)
    b3 = b_qkv.reshape(NH, 3 * HD)
    in_maps = []
    for c in range(8):
        n0, n1 = 2 * c, 2 * c + 1
        wqk = np.concatenate(
            [w3[n0, 0:128], w3[n0, 128:256], w3[n1, 0:128], w3[n1, 128:256]], axis=0)
        wv = np.concatenate([w3[n0, 256:384], w3[n1, 256:384]], axis=0)
        bqk_c = np.concatenate(
            [b3[n0, 0:128], b3[n0, 128:256], b3[n1, 0:128], b3[n1, 128:256]])
        bv_c = np.concatenate([b3[n0, 256:384], b3[n1, 256:384]])
        in_maps.append({
            "hsr": hsr.astype(ml_dtypes.bfloat16),
            "wqkT": np.ascontiguousarray(wqk.T).astype(ml_dtypes.bfloat16),
            "wvT": np.ascontiguousarray(wv.T).astype(ml_dtypes.bfloat16),
            "wdT": np.ascontiguousarray(w_dense[:, 256 * c:256 * (c + 1)].T).astype(ml_dtypes.bfloat16),
            "bqk": np.ascontiguousarray(bqk_c),
            "bvbc": np.ascontiguousarray(np.tile(bv_c[None, :], (128, 1))),
            "albt": np.ascontiguousarray(
                alibi[[n0, n1], 0, :].reshape(2, NKT, 128).transpose(2, 0, 1)),
            "mskt": mskt,
        })
    return in_maps


def run(inputs, trace=False):
    if "nc" not in _cache:
        _cache["nc"] = _build_nc()
    nc = _cache["nc"]
    in_maps = _host_prep(inputs)
    res = run_bass_kernel_spmd(nc, in_maps, list(range(8)), trace=trace)
    b_dense = np.asarray(inputs["b_dense"], dtype=np.float32)
    acc = res.results[0]["part"].astype(np.float32)
    for i in range(1, 8):
        acc = acc + res.results[i]["part"].astype(np.float32)
    out = (acc + b_dense[None, :]).reshape(B, S, H)
    return out, res.exec_time_ns


def kernel(**inputs):
    # First execution after a fresh NEFF compile has been observed to flake
    # once; run twice and return the second result.
    run(inputs, trace=False)
    out, _ = run(inputs, trace=False)
    return out
